# revision 2
# baseline (speedup 1.0000x reference)
"""Trainium2 Bass kernel for nn_AltAttention — v4.

Reference computation (B=4, S=2048, D=512, H=8, Dh=64):
    qkv  = hidden @ W_qkv + b_qkv                      -> q, k, v per head
    attn = softmax(q k^T * D**-0.5 + alibi, masked)
    out  = (attn @ v) @ W_proj + b_proj
Sharding: one head per NeuronCore; host normalizes by the softmax
denominator, sums the 8 partial projections, and adds b_proj.

v4 changes over the baseline (which was ACT-bound on 128 exp calls):
  - softmax numerator split across engines per 2-key-tile group:
      * C-groups: one DVE scalar_tensor_tensor computes
        round(s + mu + (lam*alibi - 768)) into int16, whose bf16 bit
        pattern IS exp(sigma)*2^-6 (16-bit Schraudolph; lam = 128/ln2 is
        pre-folded into W_qkv so s_ps = lam*sigma_qk).
      * A-groups: ACT exp (scale=1/lam) then a GPSIMD multiply by
        exp(alibi)*2^-6.
    No separate alibi multiply for C-groups, and the exp load on ACT
    drops ~2.7x.
  - normalization moved to host: the ones-column denominator row rides
    in the (unnormalized) fp16 x eviction, is DMA'd out per pair, and
    the host computes sum_h part_h/denom_h + b_proj. Kills the
    reciprocal + partition-broadcast + normalize-multiply tail chain.
  - q/k/v evictions stay on ACT (bias folded); x eviction on DVE (fp16,
    denominator row 0 included); proj out evictions split 3 ACT / 1 DVE.
"""

import sys

sys.path.insert(0, "/opt/trn_rl_repo")

import numpy as np
import ml_dtypes

import concourse.bass as bass
import concourse.tile as tile
from concourse import bacc, mybir
from concourse.bass_utils import run_bass_kernel_spmd
from concourse import library_config

BF16 = mybir.dt.bfloat16
F16 = mybir.dt.float16
F32 = mybir.dt.float32
I16 = mybir.dt.int16
NP_BF16 = ml_dtypes.bfloat16

B, S, D, H = 4, 2048, 512, 8
Dh = D // H  # 64
BS = B * S  # 8192
P = 128
NKT = S // P  # 16 key tiles per batch
NQB = S // 512  # 4 query blocks of 512 per batch
NSC = S // 512  # 4 s-chunks of 512 per batch (qkv phase)
SCALE = D ** (-0.5)
NG = NKT // 2  # 8 groups of 2 key tiles per query block

LAM = 128.0 / np.log(2.0)          # 184.664: bf16-Schraudolph slope
MU = 16256.0 - 7.4226              # exponent bias + sawtooth centering
PSH = 768.0                        # 6*128: scales p by 2^-6 (fp16 headroom)

# group -> path assignment (len NG). 'C' = DVE Schraudolph (alibi folded),
# 'A' = ACT exp + GPSIMD multiply, 'D' = ACT exp + DVE multiply,
# 'P' = PE identity-matmul alibi accumulate + ACT exp (no multiply).
PATHS = ("C", "A", "C", "C", "P", "C", "A", "C")
XS_ON_ACT = False  # x eviction engine: False=DVE tensor_copy, True=ACT copy
OUT_ACT = 3        # of 4 proj-out evictions per tail, how many go to ACT
DENOM_ON_GPS = False  # denominator row copy: False=DVE, True=GPSIMD


def build_program(eb: int, repeat: int = 1, paths=None, av_lag=6,
                  out_act=None):
    """Build the per-core Bass program. eb = 1 when the attention mask is
    all ones (alibi slices shared across batch), B otherwise."""
    if paths is None:
        paths = PATHS
    if out_act is None:
        out_act = OUT_ACT
    xs_on_act = XS_ON_ACT
    denom_gps = DENOM_ON_GPS
    if eb != 1:
        # masked fallback: all groups via ACT exp + mul (mask folded into ea)
        paths = ("A", "D", "A", "D", "A", "D", "A", "D")

    nc = bacc.Bacc("TRN2", target_bir_lowering=False, debug=False,
                   num_devices=H)

    hiddenT = nc.dram_tensor("hiddenT", [D, BS], BF16, kind="ExternalInput")
    # ea packs both alibi transforms in one bf16 tensor (one DMA per qb):
    # tiles for A-groups hold exp(alibi)*2^-6, tiles for C-groups hold
    # lam*alibi - 768 (bf16 rounding there adds ~1% elementwise noise that
    # the softmax normalization averages away).
    ea = nc.dram_tensor("ea", [eb, NQB, P, NKT, 512], BF16,
                        kind="ExternalInput")
    wqk = nc.dram_tensor("wqk", [4, P, P], BF16, kind="ExternalInput")
    bqk = nc.dram_tensor("bqk", [P, 1], F32, kind="ExternalInput")
    wv = nc.dram_tensor("wv", [4, P, Dh], BF16, kind="ExternalInput")
    bv = nc.dram_tensor("bv", [Dh, 1], F32, kind="ExternalInput")
    wproj = nc.dram_tensor("wproj", [Dh + 1, D], F16, kind="ExternalInput")
    ident = None
    if "P" in paths:
        ident = nc.dram_tensor("ident", [P, P], BF16, kind="ExternalInput")
    part = nc.dram_tensor("part", [BS, D], F16, kind="ExternalOutput")
    denomD = nc.dram_tensor("denom", [1, BS], F16, kind="ExternalOutput")

    hT_re = hiddenT[:].rearrange("(c p) s -> p c s", p=P)  # [128, 4, 8192]

    with tile.TileContext(nc) as tc:
        with tc.tile_pool(name="consts", bufs=1) as consts, \
             tc.tile_pool(name="persist", bufs=1) as persist:
            nc.gpsimd.load_library(library_config.proxy)
            wqk_sb = consts.tile([P, 4, P], BF16)
            nc.sync.dma_start(wqk_sb[:], wqk[:].rearrange("c p m -> p c m"))
            wv_sb = consts.tile([P, 4, Dh], BF16)
            nc.sync.dma_start(wv_sb[:], wv[:].rearrange("c p m -> p c m"))
            bqk_sb = consts.tile([P, 1], F32)
            nc.sync.dma_start(bqk_sb[:], bqk[:])
            bv_sb = consts.tile([Dh, 1], F32)
            nc.sync.dma_start(bv_sb[:], bv[:])
            wproj_sb = consts.tile([Dh + 1, D], F16)
            nc.sync.dma_start(wproj_sb[:], wproj[:])
            ident_sb = None
            if ident is not None:
                ident_sb = consts.tile([P, P], BF16)
                nc.sync.dma_start(ident_sb[:], ident[:])

            # qkT rows 0:64 = q, rows 64:128 = k; kqT is the partition swap
            qkT = persist.tile([P, BS], BF16)
            kqT = persist.tile([P, BS], BF16)
            # padded layout: tile t = [:, t, 63:128]; col 63 = ones (sums
            # row -> denominator), cols 64:128 = v^T (DMA-transpose needs
            # 128B-aligned dest offsets)
            vaug = persist.tile([P, B * NKT, P], BF16)
            nc.vector.memset(vaug[:, :, Dh - 1 : Dh], 1.0)
            vt_all = persist.tile([Dh, B * NSC, 512], BF16)
            denom_all = persist.tile([1, BS], F16)

            for rep in range(repeat):
              with tc.tile_pool(name="eapool", bufs=2) as eapool, \
                   tc.tile_pool(name="hpool", bufs=2) as hpool, \
                   tc.tile_pool(name="ppool", bufs=2) as ppool, \
                   tc.tile_pool(name="xspool", bufs=2) as xspool, \
                   tc.tile_pool(name="outpool", bufs=2) as outpool, \
                   tc.tile_pool(name="spool", bufs=2, space="PSUM") as spool, \
                   tc.tile_pool(name="vxpool", bufs=2, space="PSUM") as vxpool, \
                   tc.tile_pool(name="qops", bufs=2, space="PSUM") as qops:
                ea_prefetch = [None]
                ht_box = {}

                def p1_quanta(b):
                    """Phase-1 work for one batch as a list of emission
                    closures (interleavable into phase-2 pairs)."""
                    steps = []

                    def q_load(b=b):
                        ht = hpool.tile([P, 4, S], BF16, name="ht", tag="ht")
                        ht_box[b] = ht
                        nc.sync.dma_start(
                            ht[:], hT_re[:, :, b * S : (b + 1) * S])
                        if b == 0 and eb == 1:
                            # prefetch qb=0 alibi slice during phase 1
                            ea_prefetch[0] = eapool.tile(
                                [P, NKT, 512], BF16, name="ea_t", tag="ea")
                            nc.sync.dma_start(ea_prefetch[0][:], ea[0, 0])
                    steps.append(q_load)

                    def mk_sci(b, sci):
                        def q_sci():
                            ht = ht_box[b]
                            ssl = slice(sci * 512, (sci + 1) * 512)
                            qk_ps = qops.tile([P, 512], F32, name="qk_ps",
                                              tag="qo")
                            for c in range(4):
                                nc.tensor.matmul(qk_ps[:], wqk_sb[:, c, :],
                                                 ht[:, c, ssl],
                                                 start=(c == 0), stop=(c == 3))
                            col0 = b * S + sci * 512
                            nc.scalar.activation(
                                qkT[:, col0 : col0 + 512], qk_ps[:],
                                mybir.ActivationFunctionType.Identity,
                                bias=bqk_sb[:])
                            vt_ps = vxpool.tile([Dh + 1, 512], F32,
                                                name="vt_ps", tag="vx")
                            for c in range(4):
                                nc.tensor.matmul(vt_ps[0:Dh, :],
                                                 wv_sb[:, c, :], ht[:, c, ssl],
                                                 start=(c == 0), stop=(c == 3))
                            i = b * NSC + sci
                            nc.scalar.activation(
                                vt_all[:, i, :], vt_ps[0:Dh, :],
                                mybir.ActivationFunctionType.Identity,
                                bias=bv_sb[:])
                        return q_sci
                    steps += [mk_sci(b, sci) for sci in range(NSC)]

                    def q_post(b=b):
                        bsl = slice(b * S, (b + 1) * S)
                        # partition-swap copy: kqT = [k; q]
                        nc.sync.dma_start(kqT[0:Dh, bsl], qkT[Dh:P, bsl])
                        nc.sync.dma_start(kqT[Dh:P, bsl], qkT[0:Dh, bsl])
                        # v transpose for this batch (one XBAR-mode DMA)
                        nc.sync.dma_start(
                            vaug[:, b * NKT : (b + 1) * NKT, Dh:P],
                            vt_all[:, b * NSC : (b + 1) * NSC, :]
                            .rearrange("p a b -> p (a b)"),
                            transpose=True)
                    steps.append(q_post)
                    return steps

                # ------------- phase 2: attention + proj -------------
                for st in p1_quanta(0):
                    st()
                prev_tail = []
                p1_pend = []

                # device part layout is qb-major: row = ((qb*B + b)*4 + m)*128
                # + p; host transposes back. (b, m) merge into one contiguous
                # DMA dim this way.
                part_re = part[:].rearrange("(q b m p) d -> p q b m d",
                                            b=B, q=NQB, m=4, p=P)
                out_pair = [None]  # shared across 2 consecutive tails

                def make_tail(x_ps, qb, b):
                    """Unnormalized eviction + proj for one finished pair,
                    as emission closures interleaved into the next pair."""
                    row0 = b * S + qb * 512
                    half = b % 2
                    steps = []
                    xs_box = [None]

                    def t_xs():
                        # rows 0 = denominator, 1:65 = attn@V (fp16)
                        xs_box[0] = xspool.tile([Dh + 1, 512], F16,
                                                name="xs_t")
                        if xs_on_act:
                            nc.scalar.copy(xs_box[0][:], x_ps[:])
                        else:
                            nc.vector.tensor_copy(xs_box[0][:], x_ps[:])
                        deng = nc.gpsimd if denom_gps else nc.vector
                        deng.tensor_copy(
                            denom_all[0:1, row0 : row0 + 512],
                            xs_box[0][0:1, :])

                    steps.append(t_xs)

                    def mk_proj(m):
                        def t_proj():
                            if out_pair[0] is None:
                                out_pair[0] = outpool.tile(
                                    [P, 2, 4, 512], F16, name="out_sb")
                            out_ps = qops.tile([P, 512], F32,
                                               name="out_ps", tag="qo")
                            nc.tensor.matmul(
                                out_ps[:],
                                xs_box[0][:, m * P : (m + 1) * P],
                                wproj_sb[:], start=True, stop=True)
                            if m < out_act:
                                nc.scalar.copy(out_pair[0][:, half, m, :],
                                               out_ps[:])
                            else:
                                nc.vector.tensor_copy(
                                    out_pair[0][:, half, m, :], out_ps[:])
                            if m == 3 and half == 1:
                                nc.sync.dma_start(
                                    part_re[:, qb, b - 1 : b + 1],
                                    out_pair[0][:])
                                out_pair[0] = None
                        return t_proj

                    steps += [mk_proj(m) for m in range(4)]
                    return steps

                for qb in range(NQB):
                    if eb == 1:
                        if qb == 0 and ea_prefetch[0] is not None:
                            ea_t = ea_prefetch[0]
                        else:
                            ea_t = eapool.tile([P, NKT, 512], BF16,
                                               tag="ea", name="ea_t")
                            nc.sync.dma_start(ea_t[:], ea[0, qb])
                    for b in range(B):
                        if eb != 1:
                            ea_t = eapool.tile([P, NKT, 512], BF16,
                                               tag="ea", name="ea_t")
                            nc.sync.dma_start(ea_t[:], ea[b, qb])
                        qsl = slice(b * S + qb * 512,
                                    b * S + (qb + 1) * 512)
                        x_ps = vxpool.tile([Dh + 1, 512], F32,
                                           name="x_ps", tag="vx")
                        p_all = ppool.tile([P, NKT, 512], BF16,
                                           name="p_all")
                        if qb == 0 and b + 1 < B:
                            p1_pend.extend(p1_quanta(b + 1))

                        def scores(g):
                            s_ps = spool.tile([P, 1024], F32,
                                              name="s_ps", tag="s")
                            # row-packed pair: two K=64 matmuls running
                            # concurrently on rows 0:63 / 64:127
                            tkA, tkB = g * 2, g * 2 + 1
                            kslA = slice(b * S + tkA * P,
                                         b * S + (tkA + 1) * P)
                            kslB = slice(b * S + tkB * P,
                                         b * S + (tkB + 1) * P)
                            pe_alibi = paths[g] == "P"
                            nc.tensor.matmul(
                                s_ps[:, 0:512],
                                kqT[0:Dh, kslA], qkT[0:Dh, qsl],
                                start=True, stop=not pe_alibi)
                            nc.tensor.matmul(
                                s_ps[:, 512:1024],
                                qkT[Dh:P, kslB], kqT[Dh:P, qsl],
                                start=True, stop=not pe_alibi)
                            if pe_alibi:
                                # s += lam*alibi - 768 via identity-weight
                                # accumulating matmuls
                                nc.tensor.matmul(
                                    s_ps[:, 0:512], ident_sb[:],
                                    ea_t[:, 2 * g, :],
                                    start=False, stop=True)
                                nc.tensor.matmul(
                                    s_ps[:, 512:1024], ident_sb[:],
                                    ea_t[:, 2 * g + 1, :],
                                    start=False, stop=True)
                            return s_ps

                        def softmax(g, s_ps):
                            psl = p_all[:, 2 * g : 2 * g + 2, :]\
                                .rearrange("p a b -> p (a b)")
                            easl = ea_t[:, 2 * g : 2 * g + 2, :]\
                                .rearrange("p a b -> p (a b)")
                            if paths[g] == "C":
                                nc.vector.scalar_tensor_tensor(
                                    psl.bitcast(I16), s_ps[:], MU, easl,
                                    mybir.AluOpType.add,
                                    mybir.AluOpType.add)
                            elif paths[g] == "P":
                                # alibi already accumulated on the PE; the
                                # 1/lam scale also yields the 2^-6 shift
                                nc.scalar.activation(
                                    psl, s_ps[:],
                                    mybir.ActivationFunctionType.Exp,
                                    scale=float(1.0 / LAM))
                            else:
                                nc.scalar.activation(
                                    psl, s_ps[:],
                                    mybir.ActivationFunctionType.Exp,
                                    scale=float(1.0 / LAM))
                                eng = (nc.gpsimd if paths[g] == "A"
                                       else nc.vector)
                                eng.tensor_mul(psl, psl, easl)

                        av_n = [0]

                        def attnv(g):
                            for j in range(2):
                                tk = g * 2 + j
                                t = b * NKT + tk
                                i = av_n[0]
                                av_n[0] += 1
                                nc.tensor.matmul(
                                    x_ps[:], vaug[:, t, Dh - 1 : P],
                                    p_all[:, tk, :],
                                    start=(i == 0), stop=(i == NKT - 1))

                        pend = []
                        for g in range(NG):
                            s_ps = scores(g)
                            softmax(g, s_ps)
                            if prev_tail:
                                prev_tail.pop(0)()
                            if p1_pend:
                                p1_pend.pop(0)()
                            lag = av_lag + 2 if paths[g] == "A" else av_lag
                            pend.append((g + lag, g))
                            for slot, pg in list(pend):
                                if slot <= g + 1:
                                    attnv(pg)
                                    pend.remove((slot, pg))
                        for slot, pg in sorted(pend):
                            attnv(pg)
                        while prev_tail:
                            prev_tail.pop(0)()
                        while p1_pend:
                            p1_pend.pop(0)()

                        prev_tail = make_tail(x_ps, qb, b)
                while prev_tail:
                    prev_tail.pop(0)()
                nc.sync.dma_start(denomD[:], denom_all[:])

    nc.compile()
    return nc


_CACHE = {}


def _get_program(eb: int):
    key = ("prog", eb)
    if key not in _CACHE:
        _CACHE[key] = build_program(eb)
    return _CACHE[key]


def prepare_inputs(hidden_states, attention_mask, alibi_bias, W_qkv, b_qkv,
                   W_proj, b_proj):
    """Host-side prep: transposes, scale folding, alibi transforms, casts.
    Returns (in_maps, eb)."""
    hidden_states = np.asarray(hidden_states, dtype=np.float32)
    attention_mask = np.asarray(attention_mask)
    alibi_bias = np.asarray(alibi_bias, dtype=np.float32)
    W_qkv = np.asarray(W_qkv, dtype=np.float32)
    b_qkv = np.asarray(b_qkv, dtype=np.float32)
    W_proj = np.asarray(W_proj, dtype=np.float32)
    b_proj = np.asarray(b_proj, dtype=np.float32)

    # per-side scale: s_ps = LAM * (q.k * SCALE)
    s_side = np.float32(np.sqrt(SCALE * LAM))

    hiddenT = np.ascontiguousarray(
        hidden_states.reshape(BS, D).T).astype(NP_BF16)

    mask_trivial = bool(attention_mask.all())
    eb = 1 if mask_trivial else B
    if eb == 1:
        paths = PATHS
    else:
        paths = ("A", "D", "A", "D", "A", "D", "A", "D")
    c_tiles = np.zeros(NKT, bool)
    for g in range(NG):
        if paths[g] in ("C", "P"):
            c_tiles[2 * g : 2 * g + 2] = True

    def ea_layout(eaT):
        # eaT [S(k), S(q)] -> [NQB, 128, NKT, 512] contiguous per qb slice
        return np.ascontiguousarray(
            eaT.reshape(NKT, P, NQB, 512).transpose(2, 1, 0, 3))

    ea_all = []
    for h in range(H):
        aT = alibi_bias[0, h].T  # [S(k), S(q)]
        ea_a = np.exp(aT) * (2.0 ** -6)
        ea_c = LAM * aT - PSH
        mix = np.where(c_tiles.repeat(P)[:, None], ea_c, ea_a)
        if mask_trivial:
            ea_all.append(ea_layout(mix.astype(NP_BF16))[None])
        else:
            me = np.where(attention_mask, 1.0, 0.0)  # [B, S]
            ea_all.append(np.stack(
                [ea_layout((mix * me[bi][:, None]).astype(NP_BF16))
                 for bi in range(B)]))

    in_maps = []
    for h in range(H):
        # reference reshapes qkv to (B, S, H, 3*Dh) then splits: head h's
        # q/k/v live in columns [h*3*Dh, h*3*Dh + 3*Dh)
        qs = slice(h * 3 * Dh, h * 3 * Dh + Dh)
        ks = slice(h * 3 * Dh + Dh, h * 3 * Dh + 2 * Dh)
        vs = slice(h * 3 * Dh + 2 * Dh, h * 3 * Dh + 3 * Dh)
        wqk = np.concatenate([W_qkv[:, qs], W_qkv[:, ks]], axis=1) * s_side
        bqk_h = np.concatenate([b_qkv[qs], b_qkv[ks]]) * s_side
        wv_h = W_qkv[:, vs]
        bv_h = b_qkv[vs]
        # proj contraction row 0 multiplies the denominator row -> zero it
        wproj_aug = np.concatenate(
            [np.zeros((1, D), np.float32),
             W_proj[h * Dh : (h + 1) * Dh, :]], axis=0)
        in_maps.append({
            "hiddenT": hiddenT,
            "ea": ea_all[h],
            **({"ident": np.eye(P, dtype=NP_BF16)} if "P" in paths else {}),
            "wqk": np.ascontiguousarray(
                wqk.reshape(4, P, P).astype(NP_BF16)),
            "bqk": np.ascontiguousarray(bqk_h[:, None].astype(np.float32)),
            "wv": np.ascontiguousarray(
                wv_h.reshape(4, P, Dh).astype(NP_BF16)),
            "bv": np.ascontiguousarray(bv_h[:, None].astype(np.float32)),
            "wproj": wproj_aug.astype(np.float16),
        })
    return in_maps, eb


def kernel(**inputs):
    b_proj = np.asarray(inputs["b_proj"], dtype=np.float32)
    in_maps, eb = prepare_inputs(**inputs)
    nc = _get_program(eb)
    res = run_bass_kernel_spmd(nc, in_maps, list(range(H)))
    out = np.zeros((BS, D), np.float32)
    for h in range(H):
        # device part is qb-major: [NQB, B, 512, D] -> logical [B, NQB*512, D]
        part = res.results[h]["part"].astype(np.float32)
        part = part.reshape(NQB, B, 512, D).transpose(1, 0, 2, 3)\
            .reshape(BS, D)
        denom = res.results[h]["denom"].astype(np.float32).reshape(BS, 1)
        out += part / denom
    out += b_proj
    return out.reshape(B, S, D)
